# revision 36
# baseline (speedup 1.0000x reference)
"""Trainium2 Bass kernel for nn_CrossFrameAttentionCal (cross-frame attention).

Reference math (B=2, S=2048, DIM=1024, H=16 heads, Dh=64):
    q  = i1 @ Wq + bq                 -> [B,S,H,Dh]
    kv = i2 @ Wkv + bkv; k, v         -> [B,S,H,Dh] each
    mo = cr @ Wmo + bmo               -> [B,S,H,Dh]   (cr is [B,S,2]!)
    p  = softmax(q k^T / sqrt(Dh))    -> [B,H,S,S]
    h  = p @ v ; m = p @ mo           -> [B,S,DIM] each

Sharding: 8 cores = 2 batches x 4 head-groups (4 heads each). No collectives.

Key algebra: m = p @ (cr @ Wmo) + bmo = ((p @ cr) @ Wmo) + bmo, so the m-path
collapses to a rank-2 contraction fused into the attention matmul.

Device dataflow per core (all layouts transposed: seq on the free axis):
  qT/kT[d,i] projections from pre-transposed inputs; v[j,d] natural.
  sT[j,i] = kT^T q (PE, row-half packed per head pair)
  eT = exp(sT/8) (ScalarE, unnormalized softmax: inputs are bounded, no max
  subtraction needed; exact same math as reference softmax)
  PV: stationary [v_h | cr | ones] -> rows 0:64 h_raw^T, 64:66 w_raw^T,
  66 = den (softmax denominator) -- one PE pass computes h, the m-precursor
  AND the denominator.
  Normalize with a broadcast reciprocal; m^T = Wmo3^T @ [w_norm;1] (K=3).
Host does the input transpose/cast and output transpose (layout only).
"""

import numpy as np
import ml_dtypes

import jax
import concourse.bass as bass
import concourse.mybir as mybir
import concourse.tile as tile
from concourse import bacc
from concourse.bass2jax import (
    install_neuronx_cc_hook,
    _bass_exec_p,
    partition_id_tensor,
)

B, S, DIM, H = 2, 2048, 1024, 16
DH = 64
N_CORES = 8
HPC = 4          # heads per core
GSL = DH * HPC   # 256 output cols per core
NT_J = S // 128  # 16 j tiles
NT_C = DIM // 128  # 8 contraction tiles

# dtype config for matmul operands: all values in this problem are O(1), so
# fp16 (10-bit mantissa) is strictly better than bf16 here at the same speed.
# "fp16" | "bf16" | "f32r"
X_CFG = "fp16"

_f32 = mybir.dt.float32
_f32r = mybir.dt.float32r
_bf16 = mybir.dt.bfloat16
_EXP = mybir.ActivationFunctionType.Exp

if X_CFG == "fp16":
    X_DT, X_NP = mybir.dt.float16, np.float16
elif X_CFG == "bf16":
    X_DT, X_NP = _bf16, ml_dtypes.bfloat16
else:
    X_DT, X_NP = _f32r, np.float32
E_DT = X_DT  # exp output / PV dtype
UNPAIRED_TIMING_TEST = False


def _build_nc():
    nc = bacc.Bacc("TRN2", target_bir_lowering=False, debug=False,
                   num_devices=N_CORES)
    d = {}
    d["x1t"] = nc.dram_tensor("x1t", [DIM, S], X_DT, kind="ExternalInput").ap()
    d["x2t"] = nc.dram_tensor("x2t", [DIM, S], X_DT, kind="ExternalInput").ap()
    d["wq"] = nc.dram_tensor("wq", [DIM, GSL], X_DT, kind="ExternalInput").ap()
    d["wk"] = nc.dram_tensor("wk", [DIM, GSL], X_DT, kind="ExternalInput").ap()
    d["wv"] = nc.dram_tensor("wv", [DIM, GSL], X_DT, kind="ExternalInput").ap()
    d["bq"] = nc.dram_tensor("bq", [GSL], _f32, kind="ExternalInput").ap()
    d["bk"] = nc.dram_tensor("bk", [GSL], _f32, kind="ExternalInput").ap()
    d["bv"] = nc.dram_tensor("bv", [GSL], X_DT, kind="ExternalInput").ap()
    d["crb"] = nc.dram_tensor("crb", [S, 2], E_DT, kind="ExternalInput").ap()
    d["wmo3"] = nc.dram_tensor("wmo3", [3, GSL], X_DT, kind="ExternalInput").ap()
    d["ht"] = nc.dram_tensor("ht", [GSL, S], _f32, kind="ExternalOutput").ap()
    d["mt"] = nc.dram_tensor("mt", [GSL, S], _f32, kind="ExternalOutput").ap()
    with tile.TileContext(nc) as tc:
        _emit(nc, tc, d)
    nc.compile()
    return nc


def _emit(nc, tc, d, reps=1):
    with (
        tc.tile_pool(name="xin", bufs=1) as xin,
        tc.tile_pool(name="wgt", bufs=1) as wgt,
        tc.tile_pool(name="qkv", bufs=1) as qkv,
        tc.tile_pool(name="small", bufs=1) as small,
        tc.tile_pool(name="work", bufs=6) as work,
        tc.tile_pool(name="post", bufs=4) as post,
        tc.tile_pool(name="fin", bufs=2) as fin,
        tc.tile_pool(name="dramp", bufs=8, space="DRAM") as dramp,
        tc.tile_pool(name="psum", bufs=2, space="PSUM") as psum,
    ):
      for _rep in range(reps):
        # ---- small/weight DMAs first: they gate the first matmuls ----
        wq = wgt.tile([128, NT_C, GSL], X_DT, tag="wq")
        wk = wgt.tile([128, NT_C, GSL], X_DT, tag="wk")
        wv = wgt.tile([128, NT_C, GSL], X_DT, tag="wv")
        for name, t_ in (("wq", wq), ("wk", wk), ("wv", wv)):
            nc.sync.dma_start(t_[:], d[name].rearrange("(t p) d -> p t d", p=128))
        # ---- bulk inputs (gate the first matmuls together with weights) ----
        x1 = xin.tile([128, NT_C, S], X_DT, tag="x1")
        x2 = xin.tile([128, NT_C, S], X_DT, tag="x2")
        for t in range(NT_C):
            nc.sync.dma_start(
                x1[:, t, :], d["x1t"].rearrange("(t p) i -> p t i", p=128)[:, t, :])
            nc.sync.dma_start(
                x2[:, t, :], d["x2t"].rearrange("(t p) i -> p t i", p=128)[:, t, :])
        bq = small.tile([128, 2], _f32, tag="bq")
        bk = small.tile([128, 2], _f32, tag="bk")
        nc.sync.dma_start(bq[:], d["bq"].rearrange("(t p) -> p t", p=128))
        nc.sync.dma_start(bk[:], d["bk"].rearrange("(t p) -> p t", p=128))
        bv = small.tile([1, GSL], X_DT, tag="bv")
        nc.sync.dma_start(bv[:], d["bv"].rearrange("(o d) -> o d", o=1))
        ones1 = small.tile([1, 128], X_DT, tag="ones1")
        nc.vector.memset(ones1[:], 1.0)
        wmo3 = small.tile([3, GSL], X_DT, tag="wmo3")
        nc.sync.dma_start(wmo3[:], d["wmo3"][:])

        # PV stationary: per head [v_h(64) | cr(2) | ones(1) | pad] per j-tile
        vmc = [small.tile([128, NT_J, 68], E_DT, tag=f"vmc{h}", name=f"vmc{h}")
               for h in range(HPC)]
        for h in range(HPC):
            nc.vector.memset(vmc[h][:, :, 66:67], 1.0)
            nc.sync.dma_start(
                vmc[h][:, :, 64:66],
                d["crb"].rearrange("(t p) w -> p t w", p=128))

        qt = [qkv.tile([128, S], X_DT, tag=f"qt{p}", name=f"qt{p}")
              for p in range(2)]
        kt = [qkv.tile([128, S], X_DT, tag=f"kt{p}", name=f"kt{p}")
              for p in range(2)]

        def proj_qk(w_t, b_t, x_t, out_t, p, ptag):
            for ic in range(2):
                ps = psum.tile([128, 1024], _f32, tag=ptag, name="pps")
                for ct in range(NT_C):
                    for n in range(2):
                        sl = slice(1024 * ic + 512 * n, 1024 * ic + 512 * n + 512)
                        nc.tensor.matmul(
                            ps[:, 512 * n:512 * n + 512],
                            lhsT=w_t[:, ct, 128 * p:128 * p + 128],
                            rhs=x_t[:, ct, sl],
                            start=(ct == 0), stop=(ct == NT_C - 1))
                nc.vector.tensor_scalar_add(
                    out_t[:, 1024 * ic:1024 * ic + 1024], ps[:],
                    b_t[:, p:p + 1])

        def proj_v():
            for jt in range(NT_J):
                ps = psum.tile([128, GSL], _f32, tag="sc", name="vps")
                for ct in range(NT_C):
                    nc.tensor.matmul(ps[:], lhsT=x2[:, ct, 128 * jt:128 * jt + 128],
                                     rhs=wv[:, ct, :], start=(ct == 0), stop=False)
                nc.tensor.matmul(ps[:], lhsT=ones1[:], rhs=bv[:],
                                 start=False, stop=True)
                for h in range(HPC):
                    nc.vector.tensor_copy(vmc[h][:, jt, 0:64],
                                          ps[:, 64 * h:64 * h + 64])

        def attn(p):
            chunks = []
            for ic in range(2):
                pv = [psum.tile([128, 1024], _f32, tag="pv", name=f"pv{s}")
                      for s in range(2)]
                for jt in range(NT_J):
                    for s in range(2):
                        hl = 2 * p + s
                        sps = psum.tile([128, 1024], _f32, tag="sc", name="sps")
                        for n in range(2):
                            nc.tensor.matmul(
                                sps[:, 512 * n:512 * n + 512],
                                lhsT=kt[p][64 * s:64 * s + 64,
                                           128 * jt:128 * jt + 128],
                                rhs=qt[p][64 * s:64 * s + 64,
                                          1024 * ic + 512 * n:
                                          1024 * ic + 512 * n + 512])
                        eT = work.tile([128, 1024], E_DT, tag="e", name="eT")
                        nc.scalar.activation(eT[:], sps[:], _EXP, scale=0.125)
                        for n in range(2):
                            sl = slice(512 * n, 512 * n + 512)
                            nc.tensor.matmul(
                                pv[s][0:67, sl],
                                lhsT=vmc[hl][:, jt, 0:67],
                                rhs=eT[:, sl],
                                start=(jt == 0), stop=(jt == NT_J - 1))
                for s in range(2):
                    hl = 2 * p + s
                    praw = post.tile([67, 1024], _f32, tag="praw", name="praw")
                    nc.vector.tensor_copy(praw[:], pv[s][0:67, :])
                    db = dramp.tile([3, 1024], _f32, tag="db", name="db")
                    nc.sync.dma_start(db[:], praw[64:67, :])
                    chunks.append((hl, ic, praw, db))
            return chunks

        def finalize(chunks):
            for hl, ic, praw, db in chunks:
                rdb = fin.tile([64, 1024], _f32, tag="rdb", name="rdb")
                nc.sync.dma_start(rdb[:], db[2].partition_broadcast(64))
                rdc = fin.tile([64, 1024], _f32, tag="rdc", name="rdc")
                nc.vector.reciprocal_approx_fast(out=rdc[:], in_=rdb[:])
                hn = fin.tile([64, 1024], _f32, tag="hn", name="hn")
                nc.vector.tensor_mul(hn[:], praw[0:64, :], rdc[:])
                nc.sync.dma_start(
                    d["ht"][64 * hl:64 * hl + 64, 1024 * ic:1024 * ic + 1024],
                    hn[:])
                wnr = fin.tile([3, 1024], _f32, tag="wnr", name="wnr")
                nc.sync.dma_start(wnr[:], db[0:3])
                wn = fin.tile([3, 1024], X_DT, tag="wn", name="wn")
                nc.vector.tensor_mul(wn[:], wnr[:], rdc[0:3, :])
                mps = psum.tile([128, 1024], _f32, tag="pv", name="mps")
                for n in range(2):
                    sl = slice(512 * n, 512 * n + 512)
                    nc.tensor.matmul(mps[0:64, sl],
                                     lhsT=wmo3[:, 64 * hl:64 * hl + 64],
                                     rhs=wn[:, sl])
                mst = fin.tile([64, 1024], _f32, tag="mst", name="mst")
                nc.vector.tensor_copy(mst[:], mps[0:64, :])
                nc.sync.dma_start(
                    d["mt"][64 * hl:64 * hl + 64, 1024 * ic:1024 * ic + 1024],
                    mst[:])

        proj_qk(wq, bq, x1, qt[0], 0)
        proj_qk(wk, bk, x2, kt[0], 0)
        proj_v()
        proj_qk(wq, bq, x1, qt[1], 1)
        proj_qk(wk, bk, x2, kt[1], 1)
        c0 = attn(0)
        finalize(c0)
        c1 = attn(1)
        finalize(c1)


# ---------------------------------------------------------------------------
# host side
# ---------------------------------------------------------------------------
_CACHE = {}


def _get_runner(reps=1):
    """Build the Bass program once and wrap it in a reusable 8-core jitted fn."""
    key = ("run", reps)
    if key in _CACHE:
        return _CACHE[key]
    install_neuronx_cc_hook()
    nc = _build_nc(reps)

    pid_name = nc.partition_id_tensor.name if nc.partition_id_tensor else None
    in_names, out_names, out_avals, zero_outs = [], [], [], []
    for alloc in nc.m.functions[0].allocations:
        if not isinstance(alloc, mybir.MemoryLocationSet):
            continue
        name = alloc.memorylocations[0].name
        if alloc.kind == "ExternalInput":
            if name != pid_name:
                in_names.append(name)
        elif alloc.kind == "ExternalOutput":
            out_names.append(name)
            shape = tuple(alloc.tensor_shape)
            dtype = mybir.dt.np(alloc.dtype)
            out_avals.append(jax.core.ShapedArray(shape, dtype))
            zero_outs.append(np.zeros(shape, dtype))
    n_params = len(in_names)
    all_names = in_names + out_names
    if pid_name is not None:
        all_names = all_names + [pid_name]

    def _body(*args):
        operands = list(args)
        if pid_name is not None:
            operands.append(partition_id_tensor())
        outs = _bass_exec_p.bind(
            *operands,
            out_avals=tuple(out_avals),
            in_names=tuple(all_names),
            out_names=tuple(out_names),
            lowering_input_output_aliases=(),
            sim_require_finite=True,
            sim_require_nnan=True,
            nc=nc,
        )
        return tuple(outs)

    from jax.sharding import Mesh, PartitionSpec
    from jax.experimental.shard_map import shard_map

    devices = jax.devices()[:N_CORES]
    mesh = Mesh(np.asarray(devices), ("core",))
    donate = tuple(range(n_params, n_params + len(out_names)))
    sharded = jax.jit(
        shard_map(_body, mesh=mesh,
                  in_specs=(PartitionSpec("core"),) * (n_params + len(out_names)),
                  out_specs=(PartitionSpec("core"),) * len(out_names),
                  check_rep=False),
        donate_argnums=donate, keep_unused=True)

    def run(in_maps):
        concat_in = [
            np.concatenate([np.asarray(in_maps[c][nm]) for c in range(N_CORES)],
                           axis=0)
            for nm in in_names
        ]
        concat_zeros = [
            np.zeros((N_CORES * z.shape[0], *z.shape[1:]), z.dtype)
            for z in zero_outs
        ]
        out_arrs = sharded(*concat_in, *concat_zeros)
        return [
            {nm: np.asarray(out_arrs[i]).reshape(N_CORES, *out_avals[i].shape)[c]
             for i, nm in enumerate(out_names)}
            for c in range(N_CORES)
        ]

    _CACHE[key] = run
    _CACHE[("parts", reps)] = dict(sharded=sharded, in_names=in_names,
                                   out_names=out_names, out_avals=out_avals,
                                   zero_outs=zero_outs, n_params=n_params,
                                   mesh=mesh)
    return run


def _shard_inputs(i1, i2, cr, Wq, bq, Wkv, bkv, Wmo, bmo):
    i1 = np.asarray(i1, np.float32)
    i2 = np.asarray(i2, np.float32)
    cr = np.asarray(cr, np.float32)
    Wq = np.asarray(Wq, np.float32)
    Wkv = np.asarray(Wkv, np.float32)
    Wmo = np.asarray(Wmo, np.float32)
    bq = np.asarray(bq, np.float32)
    bkv = np.asarray(bkv, np.float32)
    bmo = np.asarray(bmo, np.float32)

    in_maps = []
    for c in range(N_CORES):
        b, g = divmod(c, N_CORES // B)
        sl = slice(GSL * g, GSL * g + GSL)
        wmo3 = np.concatenate([Wmo[:, sl], bmo[None, sl]], axis=0)
        in_maps.append({
            "x1t": np.ascontiguousarray(i1[b].T).astype(X_NP),
            "x2t": np.ascontiguousarray(i2[b].T).astype(X_NP),
            "wq": Wq[:, sl].astype(X_NP),
            "wk": Wkv[:, sl].astype(X_NP),
            "wv": Wkv[:, DIM + GSL * g:DIM + GSL * g + GSL].astype(X_NP),
            "bq": bq[sl].copy(),
            "bk": bkv[sl].copy(),
            "bv": bkv[DIM + GSL * g:DIM + GSL * g + GSL].astype(X_NP),
            "crb": cr[b].astype(X_NP),
            "wmo3": np.ascontiguousarray(wmo3).astype(X_NP),
        })
    return in_maps


def kernel(i1, i2, cr, Wq, bq, Wkv, bkv, Wmo, bmo):
    run = _get_runner()
    in_maps = _shard_inputs(i1, i2, cr, Wq, bq, Wkv, bkv, Wmo, bmo)
    results = run(in_maps)
    h = np.empty((B, S, DIM), np.float32)
    m = np.empty((B, S, DIM), np.float32)
    for c in range(N_CORES):
        b, g = divmod(c, N_CORES // B)
        sl = slice(GSL * g, GSL * g + GSL)
        h[b, :, sl] = results[c]["ht"].T
        m[b, :, sl] = results[c]["mt"].T
    return h, m
